# revision 11
# baseline (speedup 1.0000x reference)
"""Trainium2 Bass kernel for AdaptiveLogSoftmaxWithLoss (moe_routing).

Sharding across the 8 cores (all GEMMs fp8 DoubleRow):
  - tails tensor-sharded by class columns (t0: 2000/core, t1: 3840/core with
    zero-padding on core 7) over host-compacted member rows only (the
    reference masks non-member rows: ~640 rows need tail0, ~1240 tail1);
  - head sharded over (sample-tile quads x column halves): core pair
    (2j, 2j+1) owns sample tiles 4j..4j+3, even core head cols 0:2048, odd
    2048:4096;
  - both tails run two-stage on device (hidden GEMM k=1024, then a
    class-sharded logits GEMM off the fp8 hidden: t0 k=512, t1 k=256).

PSUM is a single 4-slot rotation of [128, 1024] tiles (2 banks each): every
GEMM fills at most 1024 columns per slot and every slot is drained by one
engine op, so fills pipeline 4-deep against drains instead of ping-ponging
two 2048-wide slots against 2us drains.  Per-chunk partial row-sums land in
separate accumulator columns; the host sums them.

Engine split per chunk: exact exp+accum on the scalar engine (head, t0
chunk 0, t1 A-half, t1 B-half last tile), Schraudolph exp on the DVE
(int32(x*K1+B) bitcast, mean-unbiased C) for t0 chunk 1 and the t1 B-half,
with one gpsimd fold halving each DVE accumulation.  Target logits come
from batched diagonal GEMMs (host-gathered fp8 target-weight columns in
hidden space, zeroed on non-owner cores); each batch is exp'd once into
SBUF and (iota==p)*x DVE passes extract the diagonals -- dg1/dgh extracts
are deferred into the head phase where the DVE idles.  The host recovers
each target logit as a sum of ln over cores.

Inputs arrive as a handful of concatenated per-phase blobs; b2a is split
into three dma_starts (w1t0+inp0, then the two w2t0 column chunks) so the
first hidden0 matmul starts ~3.5us in instead of waiting for the full
blob.  Host combine: sum partials over cores, subtract the exact
pad-column contributions, lse = log(sum), NLL as in the reference.  The
host only shards, compacts, gathers, quantizes and combines.
"""

import numpy as np
import ml_dtypes

import concourse.bass as bass
import concourse.bacc as bacc
import concourse.mybir as mybir
import concourse.tile as tile
from concourse.bass_utils import run_bass_kernel_spmd

BF16 = ml_dtypes.bfloat16
FP8 = ml_dtypes.float8_e4m3
H_SCALE = 8.0     # hidden cast to fp8 at 8x
W_SCALE = 64.0    # tail w2 cast to fp8 at 64x
IN_SCALE = 16.0   # inp cast to fp8 at 16x
W1_SCALE = 64.0   # w1 / head_w cast to fp8 at 64x
HID_DESCALE = 1.0 / (IN_SCALE * W1_SCALE)
DESCALE = 1.0 / (H_SCALE * W_SCALE)
NCORES = 8
N, D = 2048, 1024
H0, H1 = 512, 256
C0, C1 = 4000, 20000
HEAD = 4002
HEAD_PAD = 4096
T0 = 16000
T1 = 30257
T1_PAD = 30720
WH, W0, W1 = HEAD_PAD // 2, T0 // 8, T1_PAD // 8     # 2048, 2000, 3840
MT = N // 128                                        # 16 sample tiles
PAD_H = HEAD_PAD - HEAD   # 94 zero cols, odd cores' half
PAD_1 = T1_PAD - T1       # 463 zero cols, core 7 (all in the B-half)
KT = D // 128             # 8
HK0 = H0 // 128           # 4
HK1 = H1 // 128           # 2
W0C0 = 1024               # t0 logits chunk 0 (exact exp)
W0C1 = W0 - W0C0          # 976: t0 logits chunk 1 (Schraudolph)
HW1 = W1 // 2             # 1920: t1 A/B half width

# Schraudolph exp: exp(ps*DESCALE) ~= bitcast_f32(int32(ps*SCH_K1 + SCH_B))
SCH_C = 473120.0          # tuned for zero mean relative bias
SCH_K1 = float(np.float32((2 ** 23) / np.log(2) / 512.0))
SCH_B = float(np.float32(1065353216.0 - SCH_C))
SCH_E0 = float(np.int32(np.float32(SCH_B)).view(np.float32))  # approx exp(0)

TRACE = False
LAST_RESULT = None

_CACHED_NC = {}


def _chunks(total, step, off=0):
    out = []
    co = 0
    while co < total:
        out.append((off + co, min(step, total - co)))
        co += step
    return out


def _build_nc(P0, P1):
    N0, N1 = P0 * 128, P1 * 128
    nc = bacc.Bacc(None)
    BF = mybir.dt.bfloat16
    F8 = mybir.dt.float8e4
    F32 = mybir.dt.float32
    I32 = mybir.dt.int32
    OP = mybir.AluOpType
    ACTF = mybir.ActivationFunctionType
    DR = mybir.MatmulPerfMode.DoubleRow

    # input blobs (few DMA descriptors: the Sync/Scalar queues recycle ~8
    # DMA semaphores; many small dma_starts serialize on sem reuse)
    nb2a = KT * H0 + KT * N0 + HK0 * W0C0 + HK0 * W0C1
    nb3a = KT * N1 + KT * H1
    nb3b = HK1 * W1 + HK1 * N1 + HK0 * N0
    nb4 = KT * (512 + WH + 512)
    nra = 8 + P0 + 2 * P1
    nrv = 4 + 2 * P0 + 2 * P1
    cst_d = nc.dram_tensor("cst", [128, 129], F32, kind="ExternalInput")
    b2a_d = nc.dram_tensor("b2a", [128, nb2a], F8, kind="ExternalInput")
    b3a_d = nc.dram_tensor("b3a", [128, nb3a], F8, kind="ExternalInput")
    b3b_d = nc.dram_tensor("b3b", [128, nb3b], F8, kind="ExternalInput")
    b4_d = nc.dram_tensor("b4", [128, nb4], F8, kind="ExternalInput")
    ra_d = nc.dram_tensor("ra", [128, nra], F32, kind="ExternalOutput")
    rv_d = nc.dram_tensor("rv", [128, nrv], F32, kind="ExternalOutput")

    with tile.TileContext(nc) as tc:
        with (
            tc.tile_pool(name="const", bufs=1) as cp,
            tc.tile_pool(name="work", bufs=3) as wp,
            tc.tile_pool(name="psum", bufs=4, space="PSUM") as pp,
        ):
            cst = cp.tile([128, 129], F32)
            b2a = cp.tile([128, nb2a], F8)
            b3a = cp.tile([128, nb3a], F8)
            b3b = cp.tile([128, nb3b], F8)
            b4 = cp.tile([128, nb4], F8)
            h0T8 = cp.tile([128, HK0, N0], F8)
            h1T8 = cp.tile([128, HK1, N1], F8)
            ra = cp.tile([128, nra], F32)
            rv = cp.tile([128, nrv], F32)

            def _cut(blob, off, n, k):
                ap = blob[:, off : off + n * k]
                return ap.rearrange("p (k n) -> p k n", k=k), off + n * k

            pidx = cst[:, 0:1]
            iota = cst[:, 1:129]
            o = 0
            w1t0, o = _cut(b2a, o, H0, KT)
            inp0T, o = _cut(b2a, o, N0, KT)
            p1 = o
            w2t0c0, o = _cut(b2a, o, W0C0, HK0)
            p2 = o
            w2t0c1, o = _cut(b2a, o, W0C1, HK0)
            o = 0
            inp1T, o = _cut(b3a, o, N1, KT)
            w1t1, o = _cut(b3a, o, H1, KT)
            o = 0
            w2t1, o = _cut(b3b, o, W1, HK1)
            wgT1, o = _cut(b3b, o, N1, HK1)
            wg0h, o = _cut(b3b, o, N0, HK0)
            o = 0
            inpH, o = _cut(b4, o, 512, KT)
            hwT, o = _cut(b4, o, WH, KT)
            wgH, o = _cut(b4, o, 512, KT)

            resh = ra[:, 0:8]                               # 4 lt x 2 chunks
            res0a = ra[:, 8 : 8 + P0]                       # t0 chunk0 exact
            res1a = ra[:, 8 + P0 : 8 + P0 + 2 * P1].rearrange(
                "p (m c) -> p m c", m=P1
            )                                               # t1 A 2 chunks
            reshv = rv[:, 0:4]                              # head diag
            res0v = rv[:, 4 : 4 + P0]                       # t0 diag
            res0s = rv[:, 4 + P0 : 4 + 2 * P0]              # t0 chunk1 sch
            res1s = rv[:, 4 + 2 * P0 : 4 + 2 * P0 + P1]     # t1 B sch
            res1v = rv[:, 4 + 2 * P0 + P1 : 4 + 2 * P0 + 2 * P1]

            # loads in first-use order; b2a split so hidden0 can start on
            # piece 1 while the w2t0 column chunks stream in behind it
            nc.sync.dma_start(cst[:], cst_d[:])
            nc.sync.dma_start(b2a[:, 0:p1], b2a_d[:, 0:p1])
            nc.sync.dma_start(b2a[:, p1:p2], b2a_d[:, p1:p2])
            nc.sync.dma_start(b2a[:, p2:nb2a], b2a_d[:, p2:nb2a])
            nc.sync.dma_start(b3a[:], b3a_d[:])
            nc.sync.dma_start(b3b[:], b3b_d[:])
            nc.sync.dma_start(b4[:], b4_d[:])

            # junk tile via memset: the warmups and exp-table preload run
            # during the fixed runtime init instead of waiting for any DMA
            junk = wp.tile([128, 128], F32, tag="junk")
            nc.vector.memset(junk[:], 0.25)
            warm = wp.tile([128, 1], BF, tag="warm")
            nc.scalar.activation(warm[:], junk[:, 0:1], ACTF.Exp)

            # warm the PE HAM clock gate during init (fp32 matmuls on the
            # junk tile); the dummy DVE read frees the slot
            psw = pp.tile([128, 1024], F32, tag="big", name="psw")
            for _ in range(24):
                nc.tensor.matmul(psw[:, :128], junk, junk, start=True, stop=True)
            wsink = wp.tile([128, 1], F32, tag="wsink")
            nc.vector.tensor_scalar_mul(wsink[:], psw[:, 0:1], 0.0)

            def mm_block(ps, width, nkt, lhsT_fn, rhs_fn):
                kts = list(range(0, nkt, 2))
                for co, cw in _chunks(width, 512):
                    for ki, kt in enumerate(kts):
                        nc.tensor.matmul(
                            ps[:, co : co + cw],
                            lhsT_fn(kt),
                            rhs_fn(kt, co, cw),
                            start=(ki == 0),
                            stop=(ki == len(kts) - 1),
                            perf_mode=DR,
                        )

            def exp_drain(ps, cw, scale, s_ap):
                sc_e = wp.tile([128, 1024], BF, tag="sc_e")
                nc.scalar.activation(
                    sc_e[:, :cw], ps[:, :cw], ACTF.Exp, scale=scale, accum_out=s_ap
                )

            def schraud_drain(ps, cw, s_ap):
                # single-chunk Schraudolph: e32 on DVE, then one gpsimd op
                # folds the halves AND accumulates the row sum
                e32 = wp.tile([128, 1024], I32, tag="e32")
                nc.vector.tensor_scalar(
                    out=e32[:, :cw], in0=ps[:, :cw],
                    scalar1=SCH_K1, scalar2=SCH_B,
                    op0=OP.mult, op1=OP.add,
                )
                ef = e32[:].bitcast(F32)
                h = cw // 2
                t9 = wp.tile([128, 512], BF, tag="t9")
                nc.gpsimd.tensor_tensor(
                    out=t9[:, :h], in0=ef[:, 0:h], in1=ef[:, h : 2 * h], op=OP.add
                )
                sc2 = wp.tile([128, 512], BF, tag="sc2")
                nc.vector.tensor_scalar(
                    out=sc2[:, :h], in0=t9[:, :h],
                    scalar1=1.0, scalar2=0.0, op0=OP.mult, op1=OP.add,
                    accum_out=s_ap,
                )

            def hid_job(inT, w1, hT8, mh, co, cw):
                # hidden chunk: [128 hid rows mh] x cw samples, k=1024
                ps = pp.tile([128, 1024], F32, tag="big", name="ps")
                mm_block(
                    ps, cw, KT,
                    lambda kt: w1[:, kt : kt + 2, mh * 128 : (mh + 1) * 128],
                    lambda kt, c, w: inT[:, kt : kt + 2, co + c : co + c + w],
                )
                nc.vector.tensor_scalar_mul(
                    hT8[:, mh, co : co + cw], ps[:, :cw], HID_DESCALE * H_SCALE
                )

            def t0_job(m):
                # t0 logits off fp8 hidden0, k=512; chunk0 exact ACT exp,
                # chunk1 Schraudolph on the DVE (idle in this phase)
                ms = slice(m * 128, (m + 1) * 128)
                ps0 = pp.tile([128, 1024], F32, tag="big", name="ps0")
                mm_block(
                    ps0, W0C0, HK0,
                    lambda kt: h0T8[:, kt : kt + 2, ms],
                    lambda kt, co, cw: w2t0c0[:, kt : kt + 2, co : co + cw],
                )
                exp_drain(ps0, W0C0, DESCALE, res0a[:, m : m + 1])
                ps1 = pp.tile([128, 1024], F32, tag="big", name="ps1")
                mm_block(
                    ps1, W0C1, HK0,
                    lambda kt: h0T8[:, kt : kt + 2, ms],
                    lambda kt, co, cw: w2t0c1[:, kt : kt + 2, co : co + cw],
                )
                schraud_drain(ps1, W0C1, res0s[:, m : m + 1])

            def t1A_job(m):
                ms = slice(m * 128, (m + 1) * 128)
                for ci, (co, cw) in enumerate(_chunks(HW1, 960)):
                    ps = pp.tile([128, 1024], F32, tag="big", name="psa")
                    mm_block(
                        ps, cw, HK1,
                        lambda kt: h1T8[:, kt : kt + 2, ms],
                        lambda kt, c, w: w2t1[:, kt : kt + 2, co + c : co + c + w],
                    )
                    exp_drain(ps, cw, DESCALE, res1a[:, m, ci : ci + 1])

            def t1B_job(m):
                # both 960-chunks e32'd into one buffer, then one gpsimd op
                # folds the halves AND accumulates: one launch per m
                ms = slice(m * 128, (m + 1) * 128)
                eb = wp.tile([128, HW1], I32, tag="eb")
                for co, cw in _chunks(HW1, 960):
                    ps = pp.tile([128, 1024], F32, tag="big", name="psb")
                    mm_block(
                        ps, cw, HK1,
                        lambda kt: h1T8[:, kt : kt + 2, ms],
                        lambda kt, c, w: w2t1[
                            :, kt : kt + 2, HW1 + co + c : HW1 + co + c + w
                        ],
                    )
                    nc.vector.tensor_scalar(
                        out=eb[:, co : co + cw], in0=ps[:, :cw],
                        scalar1=SCH_K1, scalar2=SCH_B,
                        op0=OP.mult, op1=OP.add,
                    )
                ef = eb[:].bitcast(F32)
                t9b = wp.tile([128, 960], BF, tag="t9b")
                nc.gpsimd.tensor_tensor(
                    out=t9b[:], in0=ef[:, 0:960], in1=ef[:, 960:1920], op=OP.add
                )
                sc2b = wp.tile([128, 960], BF, tag="sc2b")
                nc.vector.tensor_scalar(
                    out=sc2b[:], in0=t9b[:],
                    scalar1=1.0, scalar2=0.0, op0=OP.mult, op1=OP.add,
                    accum_out=res1s[:, m : m + 1],
                )

            def head_job(lt):
                ls = slice(lt * 128, (lt + 1) * 128)
                for ci, (co, cw) in enumerate(_chunks(WH, 1024)):
                    ps = pp.tile([128, 1024], F32, tag="big", name="psh")
                    mm_block(
                        ps, cw, KT,
                        lambda kt: inpH[:, kt : kt + 2, ls],
                        lambda kt, c, w: hwT[:, kt : kt + 2, co + c : co + c + w],
                    )
                    exp_drain(ps, cw, HID_DESCALE, resh[:, lt * 2 + ci : lt * 2 + ci + 1])

            def exp_blk(ps_blk, cw, scale, tag="sc_d"):
                # diag blocks: exp into SBUF (no accum) so the PSUM slot is
                # released by ACT alone; DVE extracts lag off-path.  The host
                # recovers the logit as a sum of ln over cores (non-owner
                # cores contribute exp(0)=1).
                sc_d = wp.tile([128, 1024], F32, tag=tag)
                nc.scalar.activation(sc_d[:, :cw], ps_blk, ACTF.Exp, scale=scale)
                return sc_d

            def extract(sb_blk, t_ap):
                sc_g = wp.tile([128, 128], BF, tag="sc_g")
                nc.vector.scalar_tensor_tensor(
                    out=sc_g[:],
                    in0=iota,
                    scalar=pidx,
                    in1=sb_blk,
                    op0=OP.is_equal,
                    op1=OP.mult,
                    accum_out=t_ap,
                )

            def dg0_batch():
                # t0 target logits as diagonal GEMMs in hidden space (k=512)
                ps = pp.tile([128, 1024], F32, tag="big", name="psd0")
                for m in range(P0):
                    ms = slice(m * 128, (m + 1) * 128)
                    for ki, kt in enumerate(range(0, HK0, 2)):
                        nc.tensor.matmul(
                            ps[:, m * 128 : (m + 1) * 128],
                            h0T8[:, kt : kt + 2, ms],
                            wg0h[:, kt : kt + 2, ms],
                            start=(ki == 0), stop=(kt + 2 >= HK0),
                            perf_mode=DR,
                        )
                sd = exp_blk(ps[:, : P0 * 128], P0 * 128, DESCALE)
                for m in range(P0):
                    extract(sd[:, m * 128 : (m + 1) * 128], res0v[:, m : m + 1])

            dg1_sd = []

            def dg1_batch():
                # fills+exp only; extracts deferred into the head phase
                for lo, hi in ((0, min(8, P1)), (8, P1)):
                    if lo >= hi:
                        continue
                    ps = pp.tile([128, 1024], F32, tag="big", name="psd1")
                    for m in range(lo, hi):
                        ms = slice(m * 128, (m + 1) * 128)
                        nc.tensor.matmul(
                            ps[:, (m - lo) * 128 : (m - lo + 1) * 128],
                            h1T8[:, 0:2, ms],
                            wgT1[:, 0:2, ms],
                            start=True, stop=True, perf_mode=DR,
                        )
                    sd = exp_blk(ps[:, : (hi - lo) * 128], (hi - lo) * 128, DESCALE)
                    dg1_sd.append((sd, lo, hi))

            def dg1_extracts():
                for sd, lo, hi in dg1_sd:
                    for m in range(lo, hi):
                        extract(
                            sd[:, (m - lo) * 128 : (m - lo + 1) * 128],
                            res1v[:, m : m + 1],
                        )

            dgh_sd = []

            def dgh_batch():
                ps = pp.tile([128, 1024], F32, tag="big", name="psdh")
                for lt in range(4):
                    ls = slice(lt * 128, (lt + 1) * 128)
                    for ki, kt in enumerate(range(0, KT, 2)):
                        nc.tensor.matmul(
                            ps[:, lt * 128 : (lt + 1) * 128],
                            inpH[:, kt : kt + 2, ls],
                            wgH[:, kt : kt + 2, ls],
                            start=(ki == 0), stop=(kt + 2 >= KT),
                            perf_mode=DR,
                        )
                dgh_sd.append(exp_blk(ps[:, :512], 512, HID_DESCALE, tag="sc_dh"))

            def dgh_extracts():
                for lt in range(4):
                    extract(dgh_sd[0][:, lt * 128 : (lt + 1) * 128],
                            reshv[:, lt : lt + 1])

            # schedule: hidden0 first (its data lands first), t0 jobs with
            # hidden1 woven in, diag batches where their engines idle, t1
            # iters, then heads with the deferred extracts between them
            with nc.named_scope("main"):
                seq = [lambda mh=mh: hid_job(inp0T, w1t0, h0T8, mh, 0, N0)
                       for mh in range(HK0)]
                seq += [lambda: t0_job(0), lambda: t0_job(1)]
                for mh in range(HK1):
                    for co, cw in _chunks(N1, 1024):
                        seq.append(
                            lambda mh=mh, co=co, cw=cw: hid_job(
                                inp1T, w1t1, h1T8, mh, co, cw
                            )
                        )
                seq += [lambda m=m: t0_job(m) for m in range(2, P0)]
                seq.append(dg0_batch)
                for m in range(P1):
                    seq.append(lambda m=m: t1A_job(m))
                    seq.append(lambda m=m: t1B_job(m))
                    if m == 4:
                        seq.append(dg1_batch)
                seq += [
                    lambda: head_job(0),
                    dg1_extracts,
                    lambda: head_job(1),
                    dgh_batch,
                    lambda: head_job(2),
                    dgh_extracts,
                    lambda: head_job(3),
                ]
                for f in seq:
                    f()

            nc.sync.dma_start(ra_d[:], ra[:])
            nc.scalar.dma_start(rv_d[:], rv[:])

    nc.finalize()
    return nc


def _get_nc(P0, P1):
    key = (P0, P1)
    if key not in _CACHED_NC:
        _CACHED_NC[key] = _build_nc(P0, P1)
    return _CACHED_NC[key]


def _tiled(a2d):
    """[K, F] (K multiple of 128) -> contiguous [128, K//128, F]."""
    K, F = a2d.shape
    return np.ascontiguousarray(
        a2d.reshape(K // 128, 128, F).transpose(1, 0, 2)
    )


def _unpm(a):
    """[128, m] -> [m*128]."""
    return np.ascontiguousarray(a.T).reshape(-1)


def make_in_maps(inp, tgt, head_w, t0_w1, t0_w2, t1_w1, t1_w2):
    inp = np.asarray(inp, dtype=np.float32)
    tgt = np.asarray(tgt).astype(np.int64)

    in1 = (tgt >= C0) & (tgt < C1)
    in2 = tgt >= C1
    idx0 = np.where(in1)[0]
    idx1 = np.where(in2)[0]
    n0, n1 = len(idx0), len(idx1)
    P0 = max(1, -(-n0 // 128))
    P1 = max(1, -(-n1 // 128))
    idx0p = np.concatenate([idx0, np.zeros(P0 * 128 - n0, np.int64)])
    idx1p = np.concatenate([idx1, np.zeros(P1 * 128 - n1, np.int64)])

    inpT_s = (inp.T * IN_SCALE).astype(FP8)           # [D, N]
    inp0T = _tiled(np.ascontiguousarray(inpT_s[:, idx0p]))
    inp1T = _tiled(np.ascontiguousarray(inpT_s[:, idx1p]))
    w1t0 = _tiled((np.asarray(t0_w1, np.float32).T * W1_SCALE).astype(FP8))
    w1t1 = _tiled((np.asarray(t1_w1, np.float32).T * W1_SCALE).astype(FP8))
    w2t0_full = (np.asarray(t0_w2, np.float32).T * W_SCALE).astype(FP8)  # [H0, T0]

    hwT_full = np.zeros((D, HEAD_PAD), FP8)
    hwT_full[:, :HEAD] = (np.asarray(head_w, np.float32).T * W1_SCALE).astype(FP8)
    w2t1_full = np.zeros((H1, T1_PAD), FP8)
    w2t1_full[:, :T1] = (np.asarray(t1_w2, np.float32).T * W_SCALE).astype(FP8)

    gi = np.where(tgt < C0, tgt, np.where(tgt < C1, C0, C0 + 1))
    rel0 = tgt[idx0p] - C0
    rel1 = tgt[idx1p] - C1

    def _gathT(full, rel, own):
        # [K, osz] -> gathered [K, nrows], zeroed on non-owner cores
        g = np.ascontiguousarray(full[:, np.clip(rel, 0, full.shape[1] - 1)])
        g[:, ~own] = 0
        return _tiled(g)

    iota = np.broadcast_to(
        np.arange(128, dtype=np.float32)[None, :], (128, 128)
    ).copy()
    pidx = np.arange(128, dtype=np.float32)[:, None].copy()

    def _flat(*tiles):
        return np.ascontiguousarray(
            np.concatenate([t.reshape(128, -1) for t in tiles], axis=1)
        )

    cst = np.concatenate([pidx, iota], axis=1).astype(np.float32)
    b3a = _flat(inp1T, w1t1)
    in_maps = []
    for i in range(NCORES):
        j, h = i // 2, i % 2
        smp = slice(j * 512, (j + 1) * 512)
        gih = gi[smp]
        wgH_full = np.ascontiguousarray(hwT_full[:, gih])
        if h == 1:
            wgH_full = np.zeros_like(wgH_full)
        w2t0_i = w2t0_full[:, i * W0 : (i + 1) * W0]
        in_maps.append(
            {
                "cst": cst,
                "b2a": _flat(
                    w1t0,
                    inp0T,
                    _tiled(np.ascontiguousarray(w2t0_i[:, :W0C0])),
                    _tiled(np.ascontiguousarray(w2t0_i[:, W0C0:])),
                ),
                "b3a": b3a,
                "b3b": _flat(
                    _tiled(w2t1_full[:, i * W1 : (i + 1) * W1]),
                    _gathT(w2t1_full, rel1, (rel1 // W1) == i),
                    _gathT(w2t0_full, rel0, (rel0 // W0) == i),
                ),
                "b4": _flat(
                    _tiled(np.ascontiguousarray(inpT_s[:, smp])),
                    _tiled(hwT_full[:, h * WH : (h + 1) * WH]),
                    _tiled(wgH_full),
                ),
            }
        )
    return in_maps, tgt, (idx0, idx1, n0, n1, P0, P1)


def combine(results, tgt, meta):
    """per-core {'ra','rv'} partials -> final [N] f32 NLL."""
    idx0, idx1, n0, n1, P0, P1 = meta
    Sh = np.zeros((128, MT), np.float64)
    Th = np.zeros((128, MT), np.float64)
    S0 = np.zeros((128, P0), np.float64)
    T0s = np.zeros((128, P0), np.float64)
    S1 = np.zeros((128, P1), np.float64)
    T1s = np.zeros((128, P1), np.float64)
    for i, r in enumerate(results):
        j = i // 2
        ra = np.asarray(r["ra"], np.float64)
        rv = np.asarray(r["rv"], np.float64)
        resh = ra[:, 0:8].reshape(128, 4, 2).sum(axis=2)
        res0a = ra[:, 8 : 8 + P0]
        res1a = ra[:, 8 + P0 : 8 + P0 + 2 * P1].reshape(128, P1, 2)
        reshv = rv[:, 0:4]
        res0v = rv[:, 4 : 4 + P0]
        res0s = rv[:, 4 + P0 : 4 + 2 * P0]
        res1s = rv[:, 4 + 2 * P0 : 4 + 2 * P0 + P1]
        res1v = rv[:, 4 + 2 * P0 + P1 : 4 + 2 * P0 + 2 * P1]
        Sh[:, 4 * j : 4 * j + 4] += resh
        Th[:, 4 * j : 4 * j + 4] += np.log(reshv)
        S0 += res0a + res0s
        T0s += np.log(res0v)
        S1 += res1a[:, :, 0] + res1a[:, :, 1] + res1s
        T1s += np.log(res1v)

    # zero-padded cols: head pad on odd cores' halves (exp(0)=1 each);
    # tail1 pad all in core 7's Schraudolph B-half (approx exp(0)=SCH_E0)
    head_term = _unpm(Th) - np.log(_unpm(Sh) - PAD_H)
    lp0 = _unpm(T0s) - np.log(_unpm(S0))
    lp1 = _unpm(T1s) - np.log(_unpm(S1) - PAD_1 * SCH_E0)

    out = head_term
    out[idx0] += lp0[:n0]
    out[idx1] += lp1[:n1]
    return (-out).astype(np.float32)


def kernel(inp, tgt, head_w, t0_w1, t0_w2, t1_w1, t1_w2):
    global LAST_RESULT
    in_maps, tgt64, meta = make_in_maps(
        inp, tgt, head_w, t0_w1, t0_w2, t1_w1, t1_w2
    )
    nc = _get_nc(meta[4], meta[5])
    out = run_bass_kernel_spmd(
        nc, in_maps, core_ids=list(range(NCORES)), trace=TRACE
    )
    LAST_RESULT = out
    return combine(out.results, tgt64, meta)


# revision 22
# speedup vs baseline: 1.0024x; 1.0024x over previous
"""Trainium2 Bass kernel for AdaptiveLogSoftmaxWithLoss (moe_routing).

Sharding across the 8 cores (all GEMMs fp8 DoubleRow):
  - tails tensor-sharded by class columns (t0: 2000/core, t1: 3840/core with
    zero-padding on core 7) over host-compacted member rows only (the
    reference masks non-member rows: ~640 rows need tail0, ~1240 tail1);
  - head sharded over (sample-tile quads x column halves): core pair
    (2j, 2j+1) owns sample tiles 4j..4j+3, even core head cols 0:2048, odd
    2048:4096;
  - both tails run two-stage on device (hidden GEMM k=1024, then a
    class-sharded logits GEMM off the fp8 hidden: t0 k=512, t1 k=256).

PSUM is a single 4-slot rotation of [128, 1024] tiles (2 banks each): every
GEMM fills at most 1024 columns per slot and every slot is drained by one
engine op, so fills pipeline 4-deep against drains instead of ping-ponging
two 2048-wide slots against 2us drains.  Per-chunk partial row-sums land in
separate accumulator columns; the host sums them.

Engine split per chunk: exact exp+accum on the scalar engine (head, t0
chunk 0, t1 A-half, t1 B-half last tile), Schraudolph exp on the DVE
(int32(x*K1+B) bitcast, mean-unbiased C) for t0 chunk 1 and the t1 B-half,
with one gpsimd fold halving each DVE accumulation.  Target logits come
from batched diagonal GEMMs (host-gathered fp8 target-weight columns in
hidden space, zeroed on non-owner cores); each batch is exp'd once into
SBUF and (iota==p)*x DVE passes extract the diagonals -- dg1/dgh extracts
are deferred into the head phase where the DVE idles.  The host recovers
each target logit as a sum of ln over cores.

Inputs arrive as a handful of concatenated per-phase blobs; b2a is split
into three dma_starts (w1t0+inp0, then the two w2t0 column chunks) so the
first hidden0 matmul starts ~3.5us in instead of waiting for the full
blob.  Host combine: sum partials over cores, subtract the exact
pad-column contributions, lse = log(sum), NLL as in the reference.  The
host only shards, compacts, gathers, quantizes and combines.
"""

import numpy as np
import ml_dtypes

import concourse.bass as bass
import concourse.bacc as bacc
import concourse.mybir as mybir
import concourse.tile as tile
from concourse.bass_utils import run_bass_kernel_spmd

BF16 = ml_dtypes.bfloat16
FP8 = ml_dtypes.float8_e4m3
H_SCALE = 8.0     # hidden cast to fp8 at 8x
W_SCALE = 64.0    # tail w2 cast to fp8 at 64x
IN_SCALE = 16.0   # inp cast to fp8 at 16x
W1_SCALE = 64.0   # w1 / head_w cast to fp8 at 64x
HID_DESCALE = 1.0 / (IN_SCALE * W1_SCALE)
DESCALE = 1.0 / (H_SCALE * W_SCALE)
NCORES = 8
N, D = 2048, 1024
H0, H1 = 512, 256
C0, C1 = 4000, 20000
HEAD = 4002
HEAD_PAD = 4096
T0 = 16000
T1 = 30257
T1_PAD = 30720
WH, W0, W1 = HEAD_PAD // 2, T0 // 8, T1_PAD // 8     # 2048, 2000, 3840
MT = N // 128                                        # 16 sample tiles
PAD_H = HEAD_PAD - HEAD   # 94 zero cols, odd cores' half
PAD_1 = T1_PAD - T1       # 463 zero cols, core 7 (all in the B-half)
KT = D // 128             # 8
HK0 = H0 // 128           # 4
HK1 = H1 // 128           # 2
W0C0 = 1024               # t0 logits chunk 0 (exact exp)
W0C1 = W0 - W0C0          # 976: t0 logits chunk 1 (Schraudolph)
WA = 2048                 # t1 A half (exact ACT exp, 2x1024 chunks)
WB = W1 - WA              # 1792: t1 B half (Schraudolph, 2x896 chunks)

# Schraudolph exp: exp(ps*DESCALE) ~= bitcast_f32(int32(ps*SCH_K1 + SCH_B))
SCH_C = 473120.0          # tuned for zero mean relative bias
SCH_K1 = float(np.float32((2 ** 23) / np.log(2) / 512.0))
SCH_B = float(np.float32(1065353216.0 - SCH_C))
SCH_E0 = float(np.int32(np.float32(SCH_B)).view(np.float32))  # approx exp(0)

TRACE = False
LAST_RESULT = None

_CACHED_NC = {}


def _chunks(total, step, off=0):
    out = []
    co = 0
    while co < total:
        out.append((off + co, min(step, total - co)))
        co += step
    return out


def _build_nc(P0, P1):
    N0, N1 = P0 * 128, P1 * 128
    nc = bacc.Bacc(None)
    BF = mybir.dt.bfloat16
    F8 = mybir.dt.float8e4
    F32 = mybir.dt.float32
    I32 = mybir.dt.int32
    OP = mybir.AluOpType
    ACTF = mybir.ActivationFunctionType
    DR = mybir.MatmulPerfMode.DoubleRow

    # input blobs (few DMA descriptors: the Sync/Scalar queues recycle ~8
    # DMA semaphores; many small dma_starts serialize on sem reuse)
    nb2a = KT * H0 + KT * N0 + HK0 * W0C0 + HK0 * W0C1
    nb3a = KT * N1 + KT * H1
    nb3b = HK1 * W1 + HK1 * N1 + HK0 * N0
    nb4 = KT * (512 + WH + 512)
    nra = 8 + P0 + 2 * P1 + 2
    nrv = 4 + 2 * P0 + 2 * P1
    cst_d = nc.dram_tensor("cst", [128, 129], F32, kind="ExternalInput")
    b2a_d = nc.dram_tensor("b2a", [128, nb2a], F8, kind="ExternalInput")
    b3a_d = nc.dram_tensor("b3a", [128, nb3a], F8, kind="ExternalInput")
    b3b_d = nc.dram_tensor("b3b", [128, nb3b], F8, kind="ExternalInput")
    b4_d = nc.dram_tensor("b4", [128, nb4], F8, kind="ExternalInput")
    ra_d = nc.dram_tensor("ra", [128, nra], F32, kind="ExternalOutput")
    rv_d = nc.dram_tensor("rv", [128, nrv], F32, kind="ExternalOutput")

    with tile.TileContext(nc) as tc:
        with (
            tc.tile_pool(name="const", bufs=1) as cp,
            tc.tile_pool(name="work", bufs=3) as wp,
            tc.tile_pool(name="psum", bufs=4, space="PSUM") as pp,
        ):
            cst = cp.tile([128, 129], F32)
            b2a = cp.tile([128, nb2a], F8)
            b3a = cp.tile([128, nb3a], F8)
            b3b = cp.tile([128, nb3b], F8)
            b4 = cp.tile([128, nb4], F8)
            h0T8 = cp.tile([128, HK0, N0], F8)
            h1T8 = cp.tile([128, HK1, N1], F8)
            ra = cp.tile([128, nra], F32)
            rv = cp.tile([128, nrv], F32)

            def _cut(blob, off, n, k):
                ap = blob[:, off : off + n * k]
                return ap.rearrange("p (k n) -> p k n", k=k), off + n * k

            pidx = cst[:, 0:1]
            iota = cst[:, 1:129]
            o = 0
            w1t0, o = _cut(b2a, o, H0, KT)
            inp0T, o = _cut(b2a, o, N0, KT)
            p1 = o
            w2t0c0, o = _cut(b2a, o, W0C0, HK0)
            p2 = o
            w2t0c1, o = _cut(b2a, o, W0C1, HK0)
            o = 0
            inp1T, o = _cut(b3a, o, N1, KT)
            w1t1, o = _cut(b3a, o, H1, KT)
            o = 0
            w2t1, o = _cut(b3b, o, W1, HK1)
            wgT1, o = _cut(b3b, o, N1, HK1)
            wg0h, o = _cut(b3b, o, N0, HK0)
            o = 0
            inpH, o = _cut(b4, o, 512, KT)
            hwT, o = _cut(b4, o, WH, KT)
            wgH, o = _cut(b4, o, 512, KT)

            resh = ra[:, 0:8]                               # 4 lt x 2 chunks
            res0a = ra[:, 8 : 8 + P0]                       # t0 chunk0 exact
            res1a = ra[:, 8 + P0 : 8 + P0 + 2 * P1].rearrange(
                "p (m c) -> p m c", m=P1
            )                                               # t1 A 2 chunks
            res1bl = ra[:, 8 + P0 + 2 * P1 : 8 + P0 + 2 * P1 + 2]  # B last m
            reshv = rv[:, 0:4]                              # head diag
            res0v = rv[:, 4 : 4 + P0]                       # t0 diag
            res0s = rv[:, 4 + P0 : 4 + 2 * P0]              # t0 chunk1 sch
            res1s = rv[:, 4 + 2 * P0 : 4 + 2 * P0 + P1]     # t1 B sch
            res1v = rv[:, 4 + 2 * P0 + P1 : 4 + 2 * P0 + 2 * P1]

            # loads in first-use order; b2a split so hidden0 can start on
            # piece 1 while the w2t0 column chunks stream in behind it
            nc.sync.dma_start(cst[:], cst_d[:])
            nc.sync.dma_start(b2a[:, 0:p1], b2a_d[:, 0:p1])
            nc.sync.dma_start(b2a[:, p1:p2], b2a_d[:, p1:p2])
            nc.sync.dma_start(b2a[:, p2:nb2a], b2a_d[:, p2:nb2a])
            nc.sync.dma_start(b3a[:], b3a_d[:])
            nc.sync.dma_start(b3b[:], b3b_d[:])
            nc.sync.dma_start(b4[:], b4_d[:])

            # junk tile via memset: the warmups and exp-table preload run
            # during the fixed runtime init instead of waiting for any DMA
            junk = wp.tile([128, 128], F32, tag="junk")
            nc.vector.memset(junk[:], 0.25)
            warm = wp.tile([128, 1], BF, tag="warm")
            nc.scalar.activation(warm[:], junk[:, 0:1], ACTF.Exp)

            # warm the PE HAM clock gate during init (fp32 matmuls on the
            # junk tile); the dummy DVE read frees the slot
            psw = pp.tile([128, 1024], F32, tag="big", name="psw")
            for _ in range(24):
                nc.tensor.matmul(psw[:, :128], junk, junk, start=True, stop=True)
            wsink = wp.tile([128, 1], F32, tag="wsink")
            nc.vector.tensor_scalar_mul(wsink[:], psw[:, 0:1], 0.0)

            def mm_block(ps, width, nkt, lhsT_fn, rhs_fn, step=512):
                kts = list(range(0, nkt, 2))
                for co, cw in _chunks(width, step):
                    for ki, kt in enumerate(kts):
                        nc.tensor.matmul(
                            ps[:, co : co + cw],
                            lhsT_fn(kt),
                            rhs_fn(kt, co, cw),
                            start=(ki == 0),
                            stop=(ki == len(kts) - 1),
                            perf_mode=DR,
                        )

            def exp_drain(ps, cw, scale, s_ap):
                sc_e = wp.tile([128, 1024], BF, tag="sc_e")
                nc.scalar.activation(
                    sc_e[:, :cw], ps[:, :cw], ACTF.Exp, scale=scale, accum_out=s_ap
                )

            def schraud_drain(ps, cw, s_ap):
                # single-chunk Schraudolph: e32 on DVE, then one gpsimd op
                # folds the halves AND accumulates the row sum
                e32 = wp.tile([128, 1024], I32, tag="e32")
                nc.vector.tensor_scalar(
                    out=e32[:, :cw], in0=ps[:, :cw],
                    scalar1=SCH_K1, scalar2=SCH_B,
                    op0=OP.mult, op1=OP.add,
                )
                ef = e32[:].bitcast(F32)
                h = cw // 2
                t9 = wp.tile([128, 512], BF, tag="t9")
                nc.gpsimd.tensor_tensor(
                    out=t9[:, :h], in0=ef[:, 0:h], in1=ef[:, h : 2 * h], op=OP.add
                )
                sc2 = wp.tile([128, 512], BF, tag="sc2")
                nc.vector.tensor_scalar(
                    out=sc2[:, :h], in0=t9[:, :h],
                    scalar1=1.0, scalar2=0.0, op0=OP.mult, op1=OP.add,
                    accum_out=s_ap,
                )

            def hid_job(inT, w1, hT8, mh, co, cw):
                # hidden chunk: [128 hid rows mh] x cw samples, k=1024
                ps = pp.tile([128, 1024], F32, tag="big", name="ps")
                mm_block(
                    ps, cw, KT,
                    lambda kt: w1[:, kt : kt + 2, mh * 128 : (mh + 1) * 128],
                    lambda kt, c, w: inT[:, kt : kt + 2, co + c : co + c + w],
                )
                nc.vector.tensor_scalar_mul(
                    hT8[:, mh, co : co + cw], ps[:, :cw], HID_DESCALE * H_SCALE
                )

            def t0_job(m):
                # t0 logits off fp8 hidden0, k=512; chunk0 exact ACT exp,
                # chunk1 Schraudolph on the DVE (idle in this phase)
                ms = slice(m * 128, (m + 1) * 128)
                ps0 = pp.tile([128, 1024], F32, tag="big", name="ps0")
                mm_block(
                    ps0, W0C0, HK0,
                    lambda kt: h0T8[:, kt : kt + 2, ms],
                    lambda kt, co, cw: w2t0c0[:, kt : kt + 2, co : co + cw],
                )
                exp_drain(ps0, W0C0, DESCALE, res0a[:, m : m + 1])
                ps1 = pp.tile([128, 1024], F32, tag="big", name="ps1")
                mm_block(
                    ps1, W0C1, HK0,
                    lambda kt: h0T8[:, kt : kt + 2, ms],
                    lambda kt, co, cw: w2t0c1[:, kt : kt + 2, co : co + cw],
                )
                schraud_drain(ps1, W0C1, res0s[:, m : m + 1])

            def t1A_job(m):
                ms = slice(m * 128, (m + 1) * 128)
                for ci, (co, cw) in enumerate(_chunks(WA, 1024)):
                    ps = pp.tile([128, 1024], F32, tag="big", name="psa")
                    mm_block(
                        ps, cw, HK1,
                        lambda kt: h1T8[:, kt : kt + 2, ms],
                        lambda kt, c, w: w2t1[:, kt : kt + 2, co + c : co + c + w],
                    )
                    exp_drain(ps, cw, DESCALE, res1a[:, m, ci : ci + 1])

            def t1B_job(m):
                ms = slice(m * 128, (m + 1) * 128)
                if m == P1 - 1:
                    # last tile exact on ACT: rebalances the final iteration
                    # and keeps its pad-column correction exact
                    for ci, (co, cw) in enumerate(_chunks(WB, 896)):
                        ps = pp.tile([128, 1024], F32, tag="big", name="psb")
                        mm_block(
                            ps, cw, HK1,
                            lambda kt: h1T8[:, kt : kt + 2, ms],
                            lambda kt, c, w: w2t1[
                                :, kt : kt + 2, WA + co + c : WA + co + c + w
                            ],
                        )
                        exp_drain(ps, cw, DESCALE, res1bl[:, ci : ci + 1])
                    return
                # both 896-chunks e32'd into one buffer, then one gpsimd
                # fold of the halves and one DVE accum of the folded half
                eb = wp.tile([128, WB], I32, tag="eb")
                for co, cw in _chunks(WB, 896):
                    ps = pp.tile([128, 1024], F32, tag="big", name="psb")
                    mm_block(
                        ps, cw, HK1,
                        lambda kt: h1T8[:, kt : kt + 2, ms],
                        lambda kt, c, w: w2t1[
                            :, kt : kt + 2, WA + co + c : WA + co + c + w
                        ],
                    )
                    nc.vector.tensor_scalar(
                        out=eb[:, co : co + cw], in0=ps[:, :cw],
                        scalar1=SCH_K1, scalar2=SCH_B,
                        op0=OP.mult, op1=OP.add,
                    )
                ef = eb[:].bitcast(F32)
                t9b = wp.tile([128, 896], BF, tag="t9b")
                nc.gpsimd.tensor_tensor(
                    out=t9b[:], in0=ef[:, 0:896], in1=ef[:, 896:1792], op=OP.add
                )
                sc2b = wp.tile([128, 896], BF, tag="sc2b")
                nc.vector.tensor_scalar(
                    out=sc2b[:], in0=t9b[:],
                    scalar1=1.0, scalar2=0.0, op0=OP.mult, op1=OP.add,
                    accum_out=res1s[:, m : m + 1],
                )

            def head_job(lt):
                ls = slice(lt * 128, (lt + 1) * 128)
                for ci, (co, cw) in enumerate(_chunks(WH, 1024)):
                    ps = pp.tile([128, 1024], F32, tag="big", name="psh")
                    mm_block(
                        ps, cw, KT,
                        lambda kt: inpH[:, kt : kt + 2, ls],
                        lambda kt, c, w: hwT[:, kt : kt + 2, co + c : co + c + w],
                    )
                    exp_drain(ps, cw, HID_DESCALE, resh[:, lt * 2 + ci : lt * 2 + ci + 1])

            def exp_blk(ps_blk, cw, scale, tag="sc_d"):
                # diag blocks: exp into SBUF (no accum) so the PSUM slot is
                # released by ACT alone; DVE extracts lag off-path.  The host
                # recovers the logit as a sum of ln over cores (non-owner
                # cores contribute exp(0)=1).
                sc_d = wp.tile([128, 1024], F32, tag=tag)
                nc.scalar.activation(sc_d[:, :cw], ps_blk, ACTF.Exp, scale=scale)
                return sc_d

            def extract(sb_blk, t_ap):
                sc_g = wp.tile([128, 128], BF, tag="sc_g")
                nc.vector.scalar_tensor_tensor(
                    out=sc_g[:],
                    in0=iota,
                    scalar=pidx,
                    in1=sb_blk,
                    op0=OP.is_equal,
                    op1=OP.mult,
                    accum_out=t_ap,
                )

            def dg0_batch():
                # t0 target logits as diagonal GEMMs in hidden space (k=512);
                # FD=128 so plain matmuls + auto-FWL beat DoubleRow here
                ps = pp.tile([128, 1024], F32, tag="big", name="psd0")
                for m in range(P0):
                    ms = slice(m * 128, (m + 1) * 128)
                    for kt in range(HK0):
                        nc.tensor.matmul(
                            ps[:, m * 128 : (m + 1) * 128],
                            h0T8[:, kt, ms],
                            wg0h[:, kt, ms],
                            start=(kt == 0), stop=(kt == HK0 - 1),
                        )
                sd = exp_blk(ps[:, : P0 * 128], P0 * 128, DESCALE)
                for m in range(P0):
                    extract(sd[:, m * 128 : (m + 1) * 128], res0v[:, m : m + 1])

            dg1_sd = []

            def dg1_batch():
                # fills+exp only; extracts deferred into the head phase
                for lo, hi in ((0, min(8, P1)), (8, P1)):
                    if lo >= hi:
                        continue
                    ps = pp.tile([128, 1024], F32, tag="big", name="psd1")
                    for m in range(lo, hi):
                        ms = slice(m * 128, (m + 1) * 128)
                        for kt in range(HK1):
                            nc.tensor.matmul(
                                ps[:, (m - lo) * 128 : (m - lo + 1) * 128],
                                h1T8[:, kt, ms],
                                wgT1[:, kt, ms],
                                start=(kt == 0), stop=(kt == HK1 - 1),
                            )
                    sd = exp_blk(ps[:, : (hi - lo) * 128], (hi - lo) * 128, DESCALE)
                    dg1_sd.append((sd, lo, hi))

            def dg1_extracts():
                for sd, lo, hi in dg1_sd:
                    for m in range(lo, hi):
                        extract(
                            sd[:, (m - lo) * 128 : (m - lo + 1) * 128],
                            res1v[:, m : m + 1],
                        )

            dgh_sd = []

            def dgh_batch():
                ps = pp.tile([128, 1024], F32, tag="big", name="psdh")
                for lt in range(4):
                    ls = slice(lt * 128, (lt + 1) * 128)
                    for kt in range(KT):
                        nc.tensor.matmul(
                            ps[:, lt * 128 : (lt + 1) * 128],
                            inpH[:, kt, ls],
                            wgH[:, kt, ls],
                            start=(kt == 0), stop=(kt == KT - 1),
                        )
                dgh_sd.append(exp_blk(ps[:, :512], 512, HID_DESCALE, tag="sc_dh"))

            def dgh_extracts():
                for lt in range(4):
                    extract(dgh_sd[0][:, lt * 128 : (lt + 1) * 128],
                            reshv[:, lt : lt + 1])

            # schedule: hidden0 first (its data lands first), t0 jobs with
            # hidden1 woven in, diag batches where their engines idle, t1
            # iters, then heads with the deferred extracts between them
            with nc.named_scope("main"):
                seq = [lambda mh=mh: hid_job(inp0T, w1t0, h0T8, mh, 0, N0)
                       for mh in range(HK0)]
                seq += [lambda: t0_job(0), lambda: t0_job(1)]
                for mh in range(HK1):
                    for co, cw in _chunks(N1, 1024):
                        seq.append(
                            lambda mh=mh, co=co, cw=cw: hid_job(
                                inp1T, w1t1, h1T8, mh, co, cw
                            )
                        )
                seq += [lambda m=m: t0_job(m) for m in range(2, P0)]
                seq.append(dg0_batch)
                for m in range(P1):
                    seq.append(lambda m=m: t1A_job(m))
                    seq.append(lambda m=m: t1B_job(m))
                    if m == 4:
                        seq.append(dg1_batch)
                seq += [
                    lambda: head_job(0),
                    dg1_extracts,
                    lambda: head_job(1),
                    dgh_batch,
                    lambda: head_job(2),
                    dgh_extracts,
                    lambda: head_job(3),
                ]
                for f in seq:
                    f()

            # rv is fully written before the last head job's ra accums land:
            # issue its out-DMA first so it overlaps the final drains
            nc.scalar.dma_start(rv_d[:], rv[:])
            nc.sync.dma_start(ra_d[:], ra[:])

    nc.finalize()
    return nc


def _get_nc(P0, P1):
    key = (P0, P1)
    if key not in _CACHED_NC:
        _CACHED_NC[key] = _build_nc(P0, P1)
    return _CACHED_NC[key]


def _tiled(a2d):
    """[K, F] (K multiple of 128) -> contiguous [128, K//128, F]."""
    K, F = a2d.shape
    return np.ascontiguousarray(
        a2d.reshape(K // 128, 128, F).transpose(1, 0, 2)
    )


def _unpm(a):
    """[128, m] -> [m*128]."""
    return np.ascontiguousarray(a.T).reshape(-1)


def make_in_maps(inp, tgt, head_w, t0_w1, t0_w2, t1_w1, t1_w2):
    inp = np.asarray(inp, dtype=np.float32)
    tgt = np.asarray(tgt).astype(np.int64)

    in1 = (tgt >= C0) & (tgt < C1)
    in2 = tgt >= C1
    idx0 = np.where(in1)[0]
    idx1 = np.where(in2)[0]
    n0, n1 = len(idx0), len(idx1)
    P0 = max(1, -(-n0 // 128))
    P1 = max(1, -(-n1 // 128))
    idx0p = np.concatenate([idx0, np.zeros(P0 * 128 - n0, np.int64)])
    idx1p = np.concatenate([idx1, np.zeros(P1 * 128 - n1, np.int64)])

    inpT_s = (inp.T * IN_SCALE).astype(FP8)           # [D, N]
    inp0T = _tiled(np.ascontiguousarray(inpT_s[:, idx0p]))
    inp1T = _tiled(np.ascontiguousarray(inpT_s[:, idx1p]))
    w1t0 = _tiled((np.asarray(t0_w1, np.float32).T * W1_SCALE).astype(FP8))
    w1t1 = _tiled((np.asarray(t1_w1, np.float32).T * W1_SCALE).astype(FP8))
    w2t0_full = (np.asarray(t0_w2, np.float32).T * W_SCALE).astype(FP8)  # [H0, T0]

    hwT_full = np.zeros((D, HEAD_PAD), FP8)
    hwT_full[:, :HEAD] = (np.asarray(head_w, np.float32).T * W1_SCALE).astype(FP8)
    w2t1_full = np.zeros((H1, T1_PAD), FP8)
    w2t1_full[:, :T1] = (np.asarray(t1_w2, np.float32).T * W_SCALE).astype(FP8)

    gi = np.where(tgt < C0, tgt, np.where(tgt < C1, C0, C0 + 1))
    rel0 = tgt[idx0p] - C0
    rel1 = tgt[idx1p] - C1

    def _gathT(full, rel, own):
        # [K, osz] -> gathered [K, nrows], zeroed on non-owner cores
        g = np.ascontiguousarray(full[:, np.clip(rel, 0, full.shape[1] - 1)])
        g[:, ~own] = 0
        return _tiled(g)

    iota = np.broadcast_to(
        np.arange(128, dtype=np.float32)[None, :], (128, 128)
    ).copy()
    pidx = np.arange(128, dtype=np.float32)[:, None].copy()

    def _flat(*tiles):
        return np.ascontiguousarray(
            np.concatenate([t.reshape(128, -1) for t in tiles], axis=1)
        )

    cst = np.concatenate([pidx, iota], axis=1).astype(np.float32)
    b3a = _flat(inp1T, w1t1)
    in_maps = []
    for i in range(NCORES):
        j, h = i // 2, i % 2
        smp = slice(j * 512, (j + 1) * 512)
        gih = gi[smp]
        wgH_full = np.ascontiguousarray(hwT_full[:, gih])
        if h == 1:
            wgH_full = np.zeros_like(wgH_full)
        w2t0_i = w2t0_full[:, i * W0 : (i + 1) * W0]
        in_maps.append(
            {
                "cst": cst,
                "b2a": _flat(
                    w1t0,
                    inp0T,
                    _tiled(np.ascontiguousarray(w2t0_i[:, :W0C0])),
                    _tiled(np.ascontiguousarray(w2t0_i[:, W0C0:])),
                ),
                "b3a": b3a,
                "b3b": _flat(
                    _tiled(w2t1_full[:, i * W1 : (i + 1) * W1]),
                    _gathT(w2t1_full, rel1, (rel1 // W1) == i),
                    _gathT(w2t0_full, rel0, (rel0 // W0) == i),
                ),
                "b4": _flat(
                    _tiled(np.ascontiguousarray(inpT_s[:, smp])),
                    _tiled(hwT_full[:, h * WH : (h + 1) * WH]),
                    _tiled(wgH_full),
                ),
            }
        )
    return in_maps, tgt, (idx0, idx1, n0, n1, P0, P1)


def combine(results, tgt, meta):
    """per-core {'ra','rv'} partials -> final [N] f32 NLL."""
    idx0, idx1, n0, n1, P0, P1 = meta
    Sh = np.zeros((128, MT), np.float64)
    Th = np.zeros((128, MT), np.float64)
    S0 = np.zeros((128, P0), np.float64)
    T0s = np.zeros((128, P0), np.float64)
    S1 = np.zeros((128, P1), np.float64)
    T1s = np.zeros((128, P1), np.float64)
    for i, r in enumerate(results):
        j = i // 2
        ra = np.asarray(r["ra"], np.float64)
        rv = np.asarray(r["rv"], np.float64)
        resh = ra[:, 0:8].reshape(128, 4, 2).sum(axis=2)
        res0a = ra[:, 8 : 8 + P0]
        res1a = ra[:, 8 + P0 : 8 + P0 + 2 * P1].reshape(128, P1, 2)
        res1bl = ra[:, 8 + P0 + 2 * P1 : 8 + P0 + 2 * P1 + 2]
        reshv = rv[:, 0:4]
        res0v = rv[:, 4 : 4 + P0]
        res0s = rv[:, 4 + P0 : 4 + 2 * P0]
        res1s = rv[:, 4 + 2 * P0 : 4 + 2 * P0 + P1]
        res1v = rv[:, 4 + 2 * P0 + P1 : 4 + 2 * P0 + 2 * P1]
        Sh[:, 4 * j : 4 * j + 4] += resh
        Th[:, 4 * j : 4 * j + 4] += np.log(reshv)
        S0 += res0a + res0s
        T0s += np.log(res0v)
        # B-half sum: Schraudolph slot for m < P1-1, exact-exp pair for the
        # last tile (the complementary slots are never written on device)
        b_half = res1s.copy()
        b_half[:, P1 - 1] = res1bl[:, 0] + res1bl[:, 1]
        S1 += res1a[:, :, 0] + res1a[:, :, 1] + b_half
        T1s += np.log(res1v)

    # zero-padded cols: head pad on odd cores' halves (exp(0)=1 each);
    # tail1 pad all in core 7's B-half (approx exp(0)=SCH_E0 on Schraudolph
    # tiles, exact 1.0 on the last tile)
    head_term = _unpm(Th) - np.log(_unpm(Sh) - PAD_H)
    lp0 = _unpm(T0s) - np.log(_unpm(S0))
    padc = np.where(np.arange(P1) < P1 - 1, PAD_1 * SCH_E0, float(PAD_1))
    lp1 = _unpm(T1s) - np.log(_unpm(S1 - padc[None, :]))

    out = head_term
    out[idx0] += lp0[:n0]
    out[idx1] += lp1[:n1]
    return (-out).astype(np.float32)


def kernel(inp, tgt, head_w, t0_w1, t0_w2, t1_w1, t1_w2):
    global LAST_RESULT
    in_maps, tgt64, meta = make_in_maps(
        inp, tgt, head_w, t0_w1, t0_w2, t1_w1, t1_w2
    )
    nc = _get_nc(meta[4], meta[5])
    out = run_bass_kernel_spmd(
        nc, in_maps, core_ids=list(range(NCORES)), trace=TRACE
    )
    LAST_RESULT = out
    return combine(out.results, tgt64, meta)


# revision 25
# speedup vs baseline: 1.1111x; 1.1084x over previous
"""Trainium2 Bass kernel for AdaptiveLogSoftmaxWithLoss (moe_routing).

Sharding across the 8 cores (all GEMMs fp8 DoubleRow):
  - tails tensor-sharded by class columns (t0: 2000/core, t1: 3840/core with
    zero-padding on core 7) over host-compacted member rows only (the
    reference masks non-member rows: ~640 rows need tail0, ~1240 tail1);
  - head sharded over (sample-tile quads x column halves): core pair
    (2j, 2j+1) owns sample tiles 4j..4j+3, even core head cols 0:2048, odd
    2048:4096;
  - both tails run two-stage on device (hidden GEMM k=1024, then a
    class-sharded logits GEMM off the fp8 hidden: t0 k=512, t1 k=256).

PSUM is a single 4-slot rotation of [128, 1024] tiles (2 banks each): every
GEMM fills at most 1024 columns per slot and every slot is drained by one
engine op, so fills pipeline 4-deep against drains instead of ping-ponging
two 2048-wide slots against 2us drains.  Per-chunk partial row-sums land in
separate accumulator columns; the host sums them.

Engine split per chunk: exact exp+accum on the scalar engine (head, t0
chunk 0, t1 A-half, t1 B-half last tile), Schraudolph exp on the DVE
(int32(x*K1+B) bitcast, mean-unbiased C) for t0 chunk 1 and the t1 B-half,
with one gpsimd fold halving each DVE accumulation.  Target logits come
from batched diagonal GEMMs (host-gathered fp8 target-weight columns in
hidden space, zeroed on non-owner cores); each batch is exp'd once into
SBUF and (iota==p)*x DVE passes extract the diagonals -- dg1/dgh extracts
are deferred into the head phase where the DVE idles.  The host recovers
each target logit as a sum of ln over cores.

Inputs arrive as a handful of concatenated per-phase blobs; b2a is split
into three dma_starts (w1t0+inp0, then the two w2t0 column chunks) so the
first hidden0 matmul starts ~3.5us in instead of waiting for the full
blob.  Host combine: sum partials over cores, subtract the exact
pad-column contributions, lse = log(sum), NLL as in the reference.  The
host only shards, compacts, gathers, quantizes and combines.
"""

import numpy as np
import ml_dtypes

import concourse.bass as bass
import concourse.bacc as bacc
import concourse.mybir as mybir
import concourse.tile as tile
from concourse.bass_utils import run_bass_kernel_spmd

BF16 = ml_dtypes.bfloat16
FP8 = ml_dtypes.float8_e4m3
H_SCALE = 8.0     # hidden cast to fp8 at 8x
W_SCALE = 64.0    # tail w2 cast to fp8 at 64x
IN_SCALE = 16.0   # inp cast to fp8 at 16x
W1_SCALE = 64.0   # w1 / head_w cast to fp8 at 64x
HID_DESCALE = 1.0 / (IN_SCALE * W1_SCALE)
DESCALE = 1.0 / (H_SCALE * W_SCALE)
NCORES = 8
N, D = 2048, 1024
H0, H1 = 512, 256
C0, C1 = 4000, 20000
HEAD = 4002
HEAD_PAD = 4096
T0 = 16000
T1 = 30257
T1_PAD = 30720
WH, W0, W1 = HEAD_PAD // 2, T0 // 8, T1_PAD // 8     # 2048, 2000, 3840
MT = N // 128                                        # 16 sample tiles
PAD_H = HEAD_PAD - HEAD   # 94 zero cols, odd cores' half
PAD_1 = T1_PAD - T1       # 463 zero cols, core 7 (all in the B-half)
KT = D // 128             # 8
HK0 = H0 // 128           # 4
HK1 = H1 // 128           # 2
W0C0 = 1024               # t0 logits chunk 0 (exact exp)
W0C1 = W0 - W0C0          # 976: t0 logits chunk 1 (Schraudolph)
WA = 2048                 # t1 A half (exact ACT exp, 2x1024 chunks)
WB = W1 - WA              # 1792: t1 B half (Schraudolph, 2x896 chunks)

# Schraudolph exp: exp(ps*DESCALE) ~= bitcast_f32(int32(ps*SCH_K1 + SCH_B))
SCH_C = 473120.0          # tuned for zero mean relative bias
SCH_K1 = float(np.float32((2 ** 23) / np.log(2) / 512.0))
SCH_B = float(np.float32(1065353216.0 - SCH_C))
SCH_E0 = float(np.int32(np.float32(SCH_B)).view(np.float32))  # approx exp(0)

TRACE = False
LAST_RESULT = None

_CACHED_NC = {}


def _chunks(total, step, off=0):
    out = []
    co = 0
    while co < total:
        out.append((off + co, min(step, total - co)))
        co += step
    return out


def _build_nc(P0, P1):
    N0, N1 = P0 * 128, P1 * 128
    nc = bacc.Bacc(None)
    BF = mybir.dt.bfloat16
    F8 = mybir.dt.float8e4
    F32 = mybir.dt.float32
    I32 = mybir.dt.int32
    OP = mybir.AluOpType
    ACTF = mybir.ActivationFunctionType
    DR = mybir.MatmulPerfMode.DoubleRow

    # input blobs (few DMA descriptors: the Sync/Scalar queues recycle ~8
    # DMA semaphores; many small dma_starts serialize on sem reuse)
    nb2a = KT * H0 + KT * N0 + HK0 * W0C0 + HK0 * W0C1
    nb3a = KT * N1 + KT * H1
    nb3b = HK1 * W1 + HK1 * N1 + HK0 * N0
    nb4 = KT * (512 + WH + 512)
    nra = 8 + P0 + 2 * P1 + 2
    nrv = 4 + 2 * P0 + 2 * P1
    cst_d = nc.dram_tensor("cst", [128, 129], F32, kind="ExternalInput")
    b2a_d = nc.dram_tensor("b2a", [128, nb2a], F8, kind="ExternalInput")
    b3a_d = nc.dram_tensor("b3a", [128, nb3a], F8, kind="ExternalInput")
    b3b_d = nc.dram_tensor("b3b", [128, nb3b], F8, kind="ExternalInput")
    b4_d = nc.dram_tensor("b4", [128, nb4], F8, kind="ExternalInput")
    ra_d = nc.dram_tensor("ra", [128, nra], F32, kind="ExternalOutput")
    rv_d = nc.dram_tensor("rv", [128, nrv], F32, kind="ExternalOutput")

    with tile.TileContext(nc) as tc:
        with (
            tc.tile_pool(name="const", bufs=1) as cp,
            tc.tile_pool(name="work", bufs=3) as wp,
            tc.tile_pool(name="psum", bufs=4, space="PSUM") as pp,
        ):
            cst = cp.tile([128, 129], F32)
            b2a = cp.tile([128, nb2a], F8)
            b3a = cp.tile([128, nb3a], F8)
            b3b = cp.tile([128, nb3b], F8)
            b4 = cp.tile([128, nb4], F8)
            h0T8 = cp.tile([128, HK0, N0], F8)
            h1T8 = cp.tile([128, HK1, N1], F8)
            ra = cp.tile([128, nra], F32)
            rv = cp.tile([128, nrv], F32)

            def _cut(blob, off, n, k):
                ap = blob[:, off : off + n * k]
                return ap.rearrange("p (k n) -> p k n", k=k), off + n * k

            pidx = cst[:, 0:1]
            iota = cst[:, 1:129]
            o = 0
            w1t0, o = _cut(b2a, o, H0, KT)
            inp0T, o = _cut(b2a, o, N0, KT)
            p1 = o
            w2t0c0, o = _cut(b2a, o, W0C0, HK0)
            p2 = o
            w2t0c1, o = _cut(b2a, o, W0C1, HK0)
            o = 0
            inp1T, o = _cut(b3a, o, N1, KT)
            w1t1, o = _cut(b3a, o, H1, KT)
            o = 0
            w2t1, o = _cut(b3b, o, W1, HK1)
            wgT1, o = _cut(b3b, o, N1, HK1)
            wg0h, o = _cut(b3b, o, N0, HK0)
            o = 0
            inpH, o = _cut(b4, o, 512, KT)
            hwT, o = _cut(b4, o, WH, KT)
            wgH, o = _cut(b4, o, 512, KT)

            resh = ra[:, 0:8]                               # 4 lt x 2 chunks
            res0a = ra[:, 8 : 8 + P0]                       # t0 chunk0 exact
            res1a = ra[:, 8 + P0 : 8 + P0 + 2 * P1].rearrange(
                "p (m c) -> p m c", m=P1
            )                                               # t1 A 2 chunks
            res1bl = ra[:, 8 + P0 + 2 * P1 : 8 + P0 + 2 * P1 + 2]  # B last m
            reshv = rv[:, 0:4]                              # head diag
            res0v = rv[:, 4 : 4 + P0]                       # t0 diag
            res0s = rv[:, 4 + P0 : 4 + 2 * P0]              # t0 chunk1 sch
            res1s = rv[:, 4 + 2 * P0 : 4 + 2 * P0 + P1]     # t1 B sch
            res1v = rv[:, 4 + 2 * P0 + P1 : 4 + 2 * P0 + 2 * P1]

            # loads in first-use order; b2a split so hidden0 can start on
            # piece 1 while the w2t0 column chunks stream in behind it
            nc.sync.dma_start(cst[:], cst_d[:])
            nc.sync.dma_start(b2a[:, 0:p1], b2a_d[:, 0:p1])
            nc.sync.dma_start(b2a[:, p1:p2], b2a_d[:, p1:p2])
            nc.sync.dma_start(b2a[:, p2:nb2a], b2a_d[:, p2:nb2a])
            nc.sync.dma_start(b3a[:], b3a_d[:])
            nc.sync.dma_start(b3b[:], b3b_d[:])
            nc.sync.dma_start(b4[:], b4_d[:])

            # junk tile via memset: the warmups and exp-table preload run
            # during the fixed runtime init instead of waiting for any DMA
            junk = wp.tile([128, 128], F32, tag="junk")
            nc.vector.memset(junk[:], 0.25)
            warm = wp.tile([128, 1], BF, tag="warm")
            nc.scalar.activation(warm[:], junk[:, 0:1], ACTF.Exp)

            # warm the PE HAM clock gate during init (fp32 matmuls on the
            # junk tile); the dummy DVE read frees the slot
            psw = pp.tile([128, 1024], F32, tag="big", name="psw")
            for _ in range(24):
                nc.tensor.matmul(psw[:, :128], junk, junk, start=True, stop=True)
            wsink = wp.tile([128, 1], F32, tag="wsink")
            nc.vector.tensor_scalar_mul(wsink[:], psw[:, 0:1], 0.0)

            def mm_block(ps, width, nkt, lhsT_fn, rhs_fn, step=512):
                kts = list(range(0, nkt, 2))
                for co, cw in _chunks(width, step):
                    for ki, kt in enumerate(kts):
                        nc.tensor.matmul(
                            ps[:, co : co + cw],
                            lhsT_fn(kt),
                            rhs_fn(kt, co, cw),
                            start=(ki == 0),
                            stop=(ki == len(kts) - 1),
                            perf_mode=DR,
                        )

            # Engine queues are strict FIFO: a DVE accum that waits on a
            # gpsimd fold blocks every later DVE op.  Folds' final accums
            # are therefore deferred (software-pipelined) and flushed one
            # job later, when the fold they wait on has long finished.
            pending = []

            def flush_pending():
                while pending:
                    pending.pop(0)()

            def exp_drain(ps, cw, scale, s_ap):
                sc_e = wp.tile([128, 1024], BF, tag="sc_e")
                nc.scalar.activation(
                    sc_e[:, :cw], ps[:, :cw], ACTF.Exp, scale=scale, accum_out=s_ap
                )

            def schraud_drain(ps, cw, s_ap):
                # single-chunk Schraudolph: e32 on DVE, gpsimd fold of the
                # halves, deferred DVE accum of the folded half
                e32 = wp.tile([128, 1024], I32, tag="e32")
                nc.vector.tensor_scalar(
                    out=e32[:, :cw], in0=ps[:, :cw],
                    scalar1=SCH_K1, scalar2=SCH_B,
                    op0=OP.mult, op1=OP.add,
                )
                ef = e32[:].bitcast(F32)
                h = cw // 2
                t9 = wp.tile([128, 512], BF, tag="t9")
                nc.gpsimd.tensor_tensor(
                    out=t9[:, :h], in0=ef[:, 0:h], in1=ef[:, h : 2 * h], op=OP.add
                )

                def fin(t9=t9, h=h, s_ap=s_ap):
                    sc2 = wp.tile([128, 512], BF, tag="sc2")
                    nc.vector.tensor_scalar(
                        out=sc2[:, :h], in0=t9[:, :h],
                        scalar1=1.0, scalar2=0.0, op0=OP.mult, op1=OP.add,
                        accum_out=s_ap,
                    )

                pending.append(fin)

            def hid_job(inT, w1, hT8, mh, co, cw):
                # hidden chunk: [128 hid rows mh] x cw samples, k=1024
                ps = pp.tile([128, 1024], F32, tag="big", name="ps")
                mm_block(
                    ps, cw, KT,
                    lambda kt: w1[:, kt : kt + 2, mh * 128 : (mh + 1) * 128],
                    lambda kt, c, w: inT[:, kt : kt + 2, co + c : co + c + w],
                )
                nc.vector.tensor_scalar_mul(
                    hT8[:, mh, co : co + cw], ps[:, :cw], HID_DESCALE * H_SCALE
                )

            def t0_job(m):
                # t0 logits off fp8 hidden0, k=512; chunk0 exact ACT exp,
                # chunk1 Schraudolph on the DVE (idle in this phase)
                ms = slice(m * 128, (m + 1) * 128)
                ps0 = pp.tile([128, 1024], F32, tag="big", name="ps0")
                mm_block(
                    ps0, W0C0, HK0,
                    lambda kt: h0T8[:, kt : kt + 2, ms],
                    lambda kt, co, cw: w2t0c0[:, kt : kt + 2, co : co + cw],
                )
                exp_drain(ps0, W0C0, DESCALE, res0a[:, m : m + 1])
                ps1 = pp.tile([128, 1024], F32, tag="big", name="ps1")
                mm_block(
                    ps1, W0C1, HK0,
                    lambda kt: h0T8[:, kt : kt + 2, ms],
                    lambda kt, co, cw: w2t0c1[:, kt : kt + 2, co : co + cw],
                )
                schraud_drain(ps1, W0C1, res0s[:, m : m + 1])

            def t1A_job(m):
                ms = slice(m * 128, (m + 1) * 128)
                for ci, (co, cw) in enumerate(_chunks(WA, 1024)):
                    ps = pp.tile([128, 1024], F32, tag="big", name="psa")
                    mm_block(
                        ps, cw, HK1,
                        lambda kt: h1T8[:, kt : kt + 2, ms],
                        lambda kt, c, w: w2t1[:, kt : kt + 2, co + c : co + c + w],
                    )
                    exp_drain(ps, cw, DESCALE, res1a[:, m, ci : ci + 1])

            def t1B_job(m):
                ms = slice(m * 128, (m + 1) * 128)
                if m == P1 - 1:
                    # last tile exact on ACT: rebalances the final iteration
                    # and keeps its pad-column correction exact
                    for ci, (co, cw) in enumerate(_chunks(WB, 896)):
                        ps = pp.tile([128, 1024], F32, tag="big", name="psb")
                        mm_block(
                            ps, cw, HK1,
                            lambda kt: h1T8[:, kt : kt + 2, ms],
                            lambda kt, c, w: w2t1[
                                :, kt : kt + 2, WA + co + c : WA + co + c + w
                            ],
                        )
                        exp_drain(ps, cw, DESCALE, res1bl[:, ci : ci + 1])
                    return
                # both 896-chunks e32'd into one buffer, then one gpsimd
                # fold of the halves and one DVE accum of the folded half
                eb = wp.tile([128, WB], I32, tag="eb")
                for co, cw in _chunks(WB, 896):
                    ps = pp.tile([128, 1024], F32, tag="big", name="psb")
                    mm_block(
                        ps, cw, HK1,
                        lambda kt: h1T8[:, kt : kt + 2, ms],
                        lambda kt, c, w: w2t1[
                            :, kt : kt + 2, WA + co + c : WA + co + c + w
                        ],
                    )
                    nc.vector.tensor_scalar(
                        out=eb[:, co : co + cw], in0=ps[:, :cw],
                        scalar1=SCH_K1, scalar2=SCH_B,
                        op0=OP.mult, op1=OP.add,
                    )
                ef = eb[:].bitcast(F32)
                t9b = wp.tile([128, 896], BF, tag="t9b")
                nc.gpsimd.tensor_tensor(
                    out=t9b[:], in0=ef[:, 0:896], in1=ef[:, 896:1792], op=OP.add
                )

                def fin(t9b=t9b, m=m):
                    sc2b = wp.tile([128, 896], BF, tag="sc2b")
                    nc.vector.tensor_scalar(
                        out=sc2b[:], in0=t9b[:],
                        scalar1=1.0, scalar2=0.0, op0=OP.mult, op1=OP.add,
                        accum_out=res1s[:, m : m + 1],
                    )

                pending.append(fin)

            def head_job(lt):
                ls = slice(lt * 128, (lt + 1) * 128)
                for ci, (co, cw) in enumerate(_chunks(WH, 1024)):
                    ps = pp.tile([128, 1024], F32, tag="big", name="psh")
                    mm_block(
                        ps, cw, KT,
                        lambda kt: inpH[:, kt : kt + 2, ls],
                        lambda kt, c, w: hwT[:, kt : kt + 2, co + c : co + c + w],
                    )
                    exp_drain(ps, cw, HID_DESCALE, resh[:, lt * 2 + ci : lt * 2 + ci + 1])

            def exp_blk(ps_blk, cw, scale, tag="sc_d"):
                # diag blocks: exp into SBUF (no accum) so the PSUM slot is
                # released by ACT alone; DVE extracts lag off-path.  The host
                # recovers the logit as a sum of ln over cores (non-owner
                # cores contribute exp(0)=1).
                sc_d = wp.tile([128, 1024], F32, tag=tag)
                nc.scalar.activation(sc_d[:, :cw], ps_blk, ACTF.Exp, scale=scale)
                return sc_d

            def extract(sb_blk, t_ap):
                sc_g = wp.tile([128, 128], BF, tag="sc_g")
                nc.vector.scalar_tensor_tensor(
                    out=sc_g[:],
                    in0=iota,
                    scalar=pidx,
                    in1=sb_blk,
                    op0=OP.is_equal,
                    op1=OP.mult,
                    accum_out=t_ap,
                )

            def dg0_batch():
                # t0 target logits as diagonal GEMMs in hidden space (k=512);
                # FD=128 so plain matmuls + auto-FWL beat DoubleRow here
                ps = pp.tile([128, 1024], F32, tag="big", name="psd0")
                for m in range(P0):
                    ms = slice(m * 128, (m + 1) * 128)
                    for kt in range(HK0):
                        nc.tensor.matmul(
                            ps[:, m * 128 : (m + 1) * 128],
                            h0T8[:, kt, ms],
                            wg0h[:, kt, ms],
                            start=(kt == 0), stop=(kt == HK0 - 1),
                        )
                sd = exp_blk(ps[:, : P0 * 128], P0 * 128, DESCALE)
                for m in range(P0):
                    extract(sd[:, m * 128 : (m + 1) * 128], res0v[:, m : m + 1])

            dg1_sd = []

            def dg1_batch():
                # fills+exp only; extracts deferred into the head phase
                for lo, hi in ((0, min(8, P1)), (8, P1)):
                    if lo >= hi:
                        continue
                    ps = pp.tile([128, 1024], F32, tag="big", name="psd1")
                    for m in range(lo, hi):
                        ms = slice(m * 128, (m + 1) * 128)
                        for kt in range(HK1):
                            nc.tensor.matmul(
                                ps[:, (m - lo) * 128 : (m - lo + 1) * 128],
                                h1T8[:, kt, ms],
                                wgT1[:, kt, ms],
                                start=(kt == 0), stop=(kt == HK1 - 1),
                            )
                    sd = exp_blk(ps[:, : (hi - lo) * 128], (hi - lo) * 128, DESCALE)
                    dg1_sd.append((sd, lo, hi))

            def dg1_extracts():
                for sd, lo, hi in dg1_sd:
                    for m in range(lo, hi):
                        extract(
                            sd[:, (m - lo) * 128 : (m - lo + 1) * 128],
                            res1v[:, m : m + 1],
                        )

            dgh_sd = []

            def dgh_batch():
                ps = pp.tile([128, 1024], F32, tag="big", name="psdh")
                for lt in range(4):
                    ls = slice(lt * 128, (lt + 1) * 128)
                    for kt in range(KT):
                        nc.tensor.matmul(
                            ps[:, lt * 128 : (lt + 1) * 128],
                            inpH[:, kt, ls],
                            wgH[:, kt, ls],
                            start=(kt == 0), stop=(kt == KT - 1),
                        )
                dgh_sd.append(exp_blk(ps[:, :512], 512, HID_DESCALE, tag="sc_dh"))

            def dgh_extracts():
                for lt in range(4):
                    extract(dgh_sd[0][:, lt * 128 : (lt + 1) * 128],
                            reshv[:, lt : lt + 1])

            # schedule: hidden0 first (its data lands first), t0 jobs with
            # hidden1 woven in, diag batches where their engines idle, t1
            # iters, then heads with the deferred extracts between them
            with nc.named_scope("main"):
                seq = [lambda mh=mh: hid_job(inp0T, w1t0, h0T8, mh, 0, N0)
                       for mh in range(HK0)]
                seq += [lambda: t0_job(0), lambda: t0_job(1)]
                for mh in range(HK1):
                    for co, cw in _chunks(N1, 1024):
                        seq.append(
                            lambda mh=mh, co=co, cw=cw: hid_job(
                                inp1T, w1t1, h1T8, mh, co, cw
                            )
                        )
                seq += [lambda m=m: t0_job(m) for m in range(2, P0)]
                seq.append(dg0_batch)
                for m in range(P1):
                    seq.append(lambda m=m: t1A_job(m))
                    seq.append(lambda m=m: t1B_job(m))
                    if m == 4:
                        seq.append(dg1_batch)
                seq += [
                    lambda: head_job(0),
                    dg1_extracts,
                    lambda: head_job(1),
                    dgh_batch,
                    lambda: head_job(2),
                    dgh_extracts,
                    lambda: head_job(3),
                ]
                # run jobs, flushing deferred accums one full iteration
                # late (their gpsimd fold is certainly done by then)
                for f in seq:
                    f()
                    while len(pending) > 2:
                        pending.pop(0)()
                flush_pending()

            # rv is fully written before the last head job's ra accums land:
            # issue its out-DMA first so it overlaps the final drains
            nc.scalar.dma_start(rv_d[:], rv[:])
            nc.sync.dma_start(ra_d[:], ra[:])

    nc.finalize()
    return nc


def _get_nc(P0, P1):
    key = (P0, P1)
    if key not in _CACHED_NC:
        _CACHED_NC[key] = _build_nc(P0, P1)
    return _CACHED_NC[key]


def _tiled(a2d):
    """[K, F] (K multiple of 128) -> contiguous [128, K//128, F]."""
    K, F = a2d.shape
    return np.ascontiguousarray(
        a2d.reshape(K // 128, 128, F).transpose(1, 0, 2)
    )


def _unpm(a):
    """[128, m] -> [m*128]."""
    return np.ascontiguousarray(a.T).reshape(-1)


def make_in_maps(inp, tgt, head_w, t0_w1, t0_w2, t1_w1, t1_w2):
    inp = np.asarray(inp, dtype=np.float32)
    tgt = np.asarray(tgt).astype(np.int64)

    in1 = (tgt >= C0) & (tgt < C1)
    in2 = tgt >= C1
    idx0 = np.where(in1)[0]
    idx1 = np.where(in2)[0]
    n0, n1 = len(idx0), len(idx1)
    P0 = max(1, -(-n0 // 128))
    P1 = max(1, -(-n1 // 128))
    idx0p = np.concatenate([idx0, np.zeros(P0 * 128 - n0, np.int64)])
    idx1p = np.concatenate([idx1, np.zeros(P1 * 128 - n1, np.int64)])

    inpT_s = (inp.T * IN_SCALE).astype(FP8)           # [D, N]
    inp0T = _tiled(np.ascontiguousarray(inpT_s[:, idx0p]))
    inp1T = _tiled(np.ascontiguousarray(inpT_s[:, idx1p]))
    w1t0 = _tiled((np.asarray(t0_w1, np.float32).T * W1_SCALE).astype(FP8))
    w1t1 = _tiled((np.asarray(t1_w1, np.float32).T * W1_SCALE).astype(FP8))
    w2t0_full = (np.asarray(t0_w2, np.float32).T * W_SCALE).astype(FP8)  # [H0, T0]

    hwT_full = np.zeros((D, HEAD_PAD), FP8)
    hwT_full[:, :HEAD] = (np.asarray(head_w, np.float32).T * W1_SCALE).astype(FP8)
    w2t1_full = np.zeros((H1, T1_PAD), FP8)
    w2t1_full[:, :T1] = (np.asarray(t1_w2, np.float32).T * W_SCALE).astype(FP8)

    gi = np.where(tgt < C0, tgt, np.where(tgt < C1, C0, C0 + 1))
    rel0 = tgt[idx0p] - C0
    rel1 = tgt[idx1p] - C1

    def _gathT(full, rel, own):
        # [K, osz] -> gathered [K, nrows], zeroed on non-owner cores
        g = np.ascontiguousarray(full[:, np.clip(rel, 0, full.shape[1] - 1)])
        g[:, ~own] = 0
        return _tiled(g)

    iota = np.broadcast_to(
        np.arange(128, dtype=np.float32)[None, :], (128, 128)
    ).copy()
    pidx = np.arange(128, dtype=np.float32)[:, None].copy()

    def _flat(*tiles):
        return np.ascontiguousarray(
            np.concatenate([t.reshape(128, -1) for t in tiles], axis=1)
        )

    cst = np.concatenate([pidx, iota], axis=1).astype(np.float32)
    b3a = _flat(inp1T, w1t1)
    in_maps = []
    for i in range(NCORES):
        j, h = i // 2, i % 2
        smp = slice(j * 512, (j + 1) * 512)
        gih = gi[smp]
        wgH_full = np.ascontiguousarray(hwT_full[:, gih])
        if h == 1:
            wgH_full = np.zeros_like(wgH_full)
        w2t0_i = w2t0_full[:, i * W0 : (i + 1) * W0]
        in_maps.append(
            {
                "cst": cst,
                "b2a": _flat(
                    w1t0,
                    inp0T,
                    _tiled(np.ascontiguousarray(w2t0_i[:, :W0C0])),
                    _tiled(np.ascontiguousarray(w2t0_i[:, W0C0:])),
                ),
                "b3a": b3a,
                "b3b": _flat(
                    _tiled(w2t1_full[:, i * W1 : (i + 1) * W1]),
                    _gathT(w2t1_full, rel1, (rel1 // W1) == i),
                    _gathT(w2t0_full, rel0, (rel0 // W0) == i),
                ),
                "b4": _flat(
                    _tiled(np.ascontiguousarray(inpT_s[:, smp])),
                    _tiled(hwT_full[:, h * WH : (h + 1) * WH]),
                    _tiled(wgH_full),
                ),
            }
        )
    return in_maps, tgt, (idx0, idx1, n0, n1, P0, P1)


def combine(results, tgt, meta):
    """per-core {'ra','rv'} partials -> final [N] f32 NLL."""
    idx0, idx1, n0, n1, P0, P1 = meta
    Sh = np.zeros((128, MT), np.float64)
    Th = np.zeros((128, MT), np.float64)
    S0 = np.zeros((128, P0), np.float64)
    T0s = np.zeros((128, P0), np.float64)
    S1 = np.zeros((128, P1), np.float64)
    T1s = np.zeros((128, P1), np.float64)
    for i, r in enumerate(results):
        j = i // 2
        ra = np.asarray(r["ra"], np.float64)
        rv = np.asarray(r["rv"], np.float64)
        resh = ra[:, 0:8].reshape(128, 4, 2).sum(axis=2)
        res0a = ra[:, 8 : 8 + P0]
        res1a = ra[:, 8 + P0 : 8 + P0 + 2 * P1].reshape(128, P1, 2)
        res1bl = ra[:, 8 + P0 + 2 * P1 : 8 + P0 + 2 * P1 + 2]
        reshv = rv[:, 0:4]
        res0v = rv[:, 4 : 4 + P0]
        res0s = rv[:, 4 + P0 : 4 + 2 * P0]
        res1s = rv[:, 4 + 2 * P0 : 4 + 2 * P0 + P1]
        res1v = rv[:, 4 + 2 * P0 + P1 : 4 + 2 * P0 + 2 * P1]
        Sh[:, 4 * j : 4 * j + 4] += resh
        Th[:, 4 * j : 4 * j + 4] += np.log(reshv)
        S0 += res0a + res0s
        T0s += np.log(res0v)
        # B-half sum: Schraudolph slot for m < P1-1, exact-exp pair for the
        # last tile (the complementary slots are never written on device)
        b_half = res1s.copy()
        b_half[:, P1 - 1] = res1bl[:, 0] + res1bl[:, 1]
        S1 += res1a[:, :, 0] + res1a[:, :, 1] + b_half
        T1s += np.log(res1v)

    # zero-padded cols: head pad on odd cores' halves (exp(0)=1 each);
    # tail1 pad all in core 7's B-half (approx exp(0)=SCH_E0 on Schraudolph
    # tiles, exact 1.0 on the last tile)
    head_term = _unpm(Th) - np.log(_unpm(Sh) - PAD_H)
    lp0 = _unpm(T0s) - np.log(_unpm(S0))
    padc = np.where(np.arange(P1) < P1 - 1, PAD_1 * SCH_E0, float(PAD_1))
    lp1 = _unpm(T1s) - np.log(_unpm(S1 - padc[None, :]))

    out = head_term
    out[idx0] += lp0[:n0]
    out[idx1] += lp1[:n1]
    return (-out).astype(np.float32)


def kernel(inp, tgt, head_w, t0_w1, t0_w2, t1_w1, t1_w2):
    global LAST_RESULT
    in_maps, tgt64, meta = make_in_maps(
        inp, tgt, head_w, t0_w1, t0_w2, t1_w1, t1_w2
    )
    nc = _get_nc(meta[4], meta[5])
    out = run_bass_kernel_spmd(
        nc, in_maps, core_ids=list(range(NCORES)), trace=TRACE
    )
    LAST_RESULT = out
    return combine(out.results, tgt64, meta)
